# revision 1
# baseline (speedup 1.0000x reference)
"""Trainium2 Bass kernel for 12-head attention (B=8, N=1024, D=768). v2.

Sharding: data-parallel over batch - each of the 8 NeuronCores processes one
batch element [1024, 768] end-to-end; weights are replicated. No collectives.

Per-core algorithm (matmuls in float32r = FP22, full PE rate at N>=256):
  1. x^T via PE transposes (4 per PSUM bank, one batched copy), interleaved
     per seq-tile with the V matmuls.
  2. Q^T, K^T = w-as-lhsT @ x^T -> [768, 1024] each. V stored bf16 with a
     ones column per head: V'[:, 65h:65h+64] = V_h, V'[:, 65h+64] = 1.
  3. Per head pair pr, per key tile kt: 4 S matmuls (A0,B0,B1,A1 order so the
     K=64 row-group pairs can overlap on HW), each into its own [128,512]
     PSUM tile. exp: A tiles on ScalarE (exact, ->bf16), B tiles on DVE via
     the Schraudolph bit trick (x*A+B -> int16 == bf16 bits of e^x, one
     tensor_scalar op). O~'[h,qb] += V'_h[kt]-as-lhsT @ P~^T -> PSUM [65,512]
     (rows 0-63 unnormalized O^T head rows, row 64 softmax denominators).
  4. Epilogue per pr: stage [65,512] tiles to SBUF (split ScalarE/DVE), DMA
     rows 0-63 into ot_sb and row 64 into spair; reciprocal on DVE; 1/s
     broadcast to 64 partitions via K=1 matmuls into psA tiles at the next
     pair's start (deferred 2 pairs so nothing stalls), multiplied into
     ot_sb on DVE; last pair uses a qb-split fast tail to unblock proj.
  5. out = O^T-as-lhsT @ w_proj -> [1024, 768] -> HBM.

Biases enter as K=1 matmuls appended to each accumulation group (skipped
when the host sees all-zero biases, which is what the reference generates).
"""

import os
import numpy as np

import concourse.bass as bass
from concourse import bacc
import concourse.mybir as mybir
import concourse.tile as tile
from concourse.masks import make_identity

F32 = mybir.dt.float32
F32R = mybir.dt.float32r
BF16 = mybir.dt.bfloat16
I16 = mybir.dt.int16
AF = mybir.ActivationFunctionType
ALU = mybir.AluOpType

N = 1024   # sequence length
D = 768    # model dim
H = 12     # heads
HD = 64    # head dim
NT = N // 128   # 8 seq tiles
DT = D // 128   # 6 dim tiles
NP = H // 2     # 6 head pairs
SCALE = HD ** -0.5  # 0.125
VPW = H * (HD + 1)  # 780: per-head 64 V cols + ones col

# Schraudolph fast-exp in bf16 bit space: bits = int16(x*EXP_A + EXP_B),
# bitcast to bf16 ~= e^(SCALE*x).  (2^7 * log2 e * SCALE, 127 * 2^7)
EXP_A = 184.66496523378733 * SCALE
EXP_B = 16256.0


def _r(ap):
    """Reinterpret an fp32 AP as float32r for full-rate PE matmuls."""
    return ap.bitcast(F32R)


def build_module(with_bias: bool, loop_iters: int = 0) -> bass.Bass:
    nc = bacc.Bacc("TRN2", target_bir_lowering=False, debug=False)

    x_d = nc.dram_tensor("x", [N, D], F32, kind="ExternalInput")
    wqkv_d = nc.dram_tensor("w_qkv", [D, 3 * D], F32, kind="ExternalInput")
    bqkv_d = nc.dram_tensor("b_qkv", [1, 3 * D], F32, kind="ExternalInput")
    wp_d = nc.dram_tensor("w_proj", [D, D], F32, kind="ExternalInput")
    bp_d = nc.dram_tensor("b_proj", [1, D], F32, kind="ExternalInput")
    out_d = nc.dram_tensor("out", [N, D], F32, kind="ExternalOutput")

    with tile.TileContext(nc) as tc:
        if loop_iters:
            with tc.For_i(0, loop_iters, 1, hint_engines=(mybir.EngineType.PE,)):
                _emit(nc, tc, x_d, wqkv_d, bqkv_d, wp_d, bp_d, out_d,
                      with_bias)
        else:
            _emit(nc, tc, x_d, wqkv_d, bqkv_d, wp_d, bp_d, out_d, with_bias)
    nc.compile()
    return nc


def _emit(nc, tc, x_d, wqkv_d, bqkv_d, wp_d, bp_d, out_d, with_bias):
    # ---- persistent pools / tensors; big weight DMAs issued first ----
    top = tc.alloc_tile_pool(name="top", bufs=1)
    identity = top.tile([128, 128], F32, name="identity")
    make_identity(nc, identity)
    ones = top.tile([1, 512], F32, name="ones")
    nc.gpsimd.memset(ones, 1.0)

    qt_sb = top.tile([128, DT, N], F32R, name="qt_sb")    # Q^T [768, 1024]
    kt_sb = top.tile([128, DT, N], F32R, name="kt_sb")    # K^T [768, 1024]
    vp_sb = top.tile([128, NT, VPW], BF16, name="vp_sb")  # V' bf16
    ot_sb = top.tile([128, DT, N], F32R, name="ot_sb")    # O^T [768, 1024]

    if with_bias:
        bq_row = top.tile([1, 3 * D], F32, name="bq_row")
        bp_row = top.tile([1, D], F32, name="bp_row")
        nc.scalar.dma_start(bq_row, bqkv_d.ap())
        nc.scalar.dma_start(bp_row, bp_d.ap())
    else:
        bq_row = bp_row = None

    # Weights go through ScalarE's HWDGE queue so the x-tile loads on SP's
    # queue aren't stuck behind 7 MB of weight traffic.
    xt_pool = tc.alloc_tile_pool(name="xtp", bufs=1)
    xt_sb = xt_pool.tile([128, DT, N], F32R, name="xt_sb")  # x^T [768, 1024]

    wv_pool0 = tc.alloc_tile_pool(name="wv", bufs=1)
    wv_sb = wv_pool0.tile([128, DT, D], F32R, name="wv_sb")
    for voff, vw in ((0, 512), (512, 256)):
        nc.scalar.dma_start(
            wv_sb[:, :, voff:voff + vw],
            wqkv_d.ap()[:, 2 * D + voff:2 * D + voff + vw].rearrange(
                "(ko p) n -> p ko n", p=128).bitcast(F32R))

    wqk_pool = tc.alloc_tile_pool(name="wqk", bufs=1)
    wqk_sb = wqk_pool.tile([128, DT, 2 * D], F32R, name="wqk_sb")
    for wh in range(2):  # w_q then w_k, so the Q matmuls can start sooner
        nc.scalar.dma_start(
            wqk_sb[:, :, wh * D:(wh + 1) * D],
            wqkv_d.ap()[:, wh * D:(wh + 1) * D].rearrange(
                "(ko p) n -> p ko n", p=128).bitcast(F32R))

    xs_pool = tc.alloc_tile_pool(name="xs", bufs=8)

    psA = tc.alloc_tile_pool(name="psA", bufs=4, space="PSUM")
    psB = tc.alloc_tile_pool(name="psB", bufs=4, space="PSUM")

    vpv = vp_sb.rearrange("p st (h c) -> p st h c", c=HD + 1)
    nc.gpsimd.memset(vpv[:, :, :, HD:HD + 1], 1.0)  # per-head ones col

    exp_warm = top.tile([1, 8], F32, name="exp_warm")
    nc.scalar.activation(exp_warm, ones[0:1, 0:8], AF.Exp, scale=1.0)

    # ---- phase 1: x^T transposes interleaved with V matmuls, per seq tile --
    def emit_v(st):
        for nb, (noff, nw) in enumerate(((0, 512), (512, 256))):
            ps = psB.tile([128, 512], F32, tag="o", name=f"v_{st}_{nb}")
            seg = ps[:, 0:nw]
            for kt_i in range(DT):
                nc.tensor.matmul(
                    seg,
                    xt_sb[:, kt_i, st * 128:(st + 1) * 128],
                    wv_sb[:, kt_i, noff:noff + nw],
                    start=(kt_i == 0),
                    stop=(kt_i == DT - 1 and not with_bias),
                )
            if with_bias:
                nc.tensor.matmul(
                    seg,
                    ones[0:1, 0:128],
                    bq_row[0:1, 2 * D + noff:2 * D + noff + nw],
                    start=False, stop=True,
                )
            h0, hn = noff // HD, nw // HD
            nc.vector.tensor_copy(
                vpv[:, st, h0:h0 + hn, 0:HD],
                seg.rearrange("p (h c) -> p h c", c=HD),
            )

    x_ts = []
    for st in range(NT):
        x_t = xs_pool.tile([128, D], F32, tag="xrow", name=f"x_{st}")
        nc.sync.dma_start(x_t, x_d.ap()[st * 128:(st + 1) * 128, :])
        x_ts.append(x_t)
    for st in range(NT):
        x_t = x_ts[st]
        for half, (d0, dn) in enumerate(((0, 4), (4, 2))):
            pt = psA.tile([128, 512], F32, tag="s", name=f"pt_{st}_{half}")
            for i in range(dn):
                nc.tensor.transpose(
                    pt[:, i * 128:(i + 1) * 128],
                    x_t[:, (d0 + i) * 128:(d0 + i + 1) * 128],
                    identity)
            nc.scalar.copy(
                xt_sb[:, d0:d0 + dn, st * 128:(st + 1) * 128],
                pt[:, 0:dn * 128].rearrange("p (d c) -> p d c", c=128))
        if st >= 3:
            emit_v(st - 3)  # V lags three tiles: overlap + wv DMA arrival
    for st in range(NT - 3, NT):
        emit_v(st)

    xs_pool.release()

    # ---- phase 2: Q^T / K^T ----
    def emit_qk(mt):
        for which, dst in ((0, qt_sb), (1, kt_sb)):
            for qb in range(2):
                ps = psA.tile([128, 512], F32, tag="s",
                              name=f"qk_{which}_{mt}_{qb}")
                for kt_i in range(DT):
                    nc.tensor.matmul(
                        ps,
                        wqk_sb[:, kt_i, which * D + mt * 128:
                               which * D + (mt + 1) * 128],
                        xt_sb[:, kt_i, qb * 512:(qb + 1) * 512],
                        start=(kt_i == 0),
                        stop=(kt_i == DT - 1 and not with_bias),
                    )
                if with_bias:
                    nc.tensor.matmul(
                        ps,
                        bq_row[0:1, which * D + mt * 128:
                               which * D + (mt + 1) * 128],
                        ones[0:1, 0:512],
                        start=False, stop=True,
                    )
                nc.scalar.copy(dst[:, mt, qb * 512:(qb + 1) * 512], ps)

    for mt in range(DT):
        emit_qk(mt)
    wqk_pool.release()
    wv_pool0.release()
    xt_pool.release()

    # ---- phase 3: attention, head pairs ----
    late = tc.alloc_tile_pool(name="late", bufs=1)
    wp_sb = late.tile([128, DT, D], F32R, name="wp_sb")
    nc.sync.dma_start(
        wp_sb, wp_d.ap().rearrange("(ko p) n -> p ko n", p=128).bitcast(F32R))
    # Per-pair softmax-denominator and reciprocal tiles (partitions 0-1).
    spair = [late.tile([2, N], F32, name=f"spair_{p}") for p in range(NP)]
    rpair = [late.tile([2, N], F32, name=f"rpair_{p}") for p in range(NP)]
    pexp_pool = tc.alloc_tile_pool(name="pexp", bufs=10)
    stage_pool = tc.alloc_tile_pool(name="stage", bufs=4)
    flat_pool = tc.alloc_tile_pool(name="flat", bufs=3)

    def emit_norm(pr, fl):
        # Broadcast 1/s to the 64 head rows via K=1 matmuls into psA tiles
        # (allocated before the next pair's S tiles), multiply on DVE.
        for qb in range(2):
            qs = slice(qb * 512, (qb + 1) * 512)
            r_ps = psA.tile([128, 512], F32, tag="s", name=f"rps_{pr}_{qb}")
            nc.tensor.matmul(r_ps[0:64, :], ones[0:1, 0:HD],
                             rpair[pr][0:1, qs], start=True, stop=True)
            nc.tensor.matmul(r_ps[64:128, :], ones[0:1, 0:HD],
                             fl[0:1, qs], start=True, stop=True)
            dst = ot_sb[:, pr, qs]
            nc.vector.tensor_mul(out=dst, in0=dst, in1=r_ps)

    pending_norm = []
    for pr in range(NP):  # heads (2*pr, 2*pr+1); Q/K tile mt = pr
        if len(pending_norm) >= 2:  # norm lags 2 pairs so recip is long done
            emit_norm(*pending_norm.pop(0))
        o_ps = {}
        for hh in range(2):
            for qb in range(2):
                o_ps[(hh, qb)] = psB.tile(
                    [65, 512], F32, tag="o", name=f"o_{pr}_{hh}_{qb}")

        def emit_o(kt_i, pexp):
            for hh, qb in ((0, 0), (0, 1), (1, 0), (1, 1)):
                h = 2 * pr + hh
                nc.tensor.matmul(
                    o_ps[(hh, qb)],
                    vp_sb[:, kt_i, h * (HD + 1):(h + 1) * (HD + 1)],
                    pexp[(hh, qb)],
                    start=(kt_i == 0),
                    stop=(kt_i == NT - 1),
                    skip_group_check=True,
                )

        prev = None  # (kt_i, pexp) pending O accumulation - one kt behind
        for kt_i in range(NT):
            kblk = slice(kt_i * 128, (kt_i + 1) * 128)
            s_t = {}
            for hh, qb in ((0, 0), (1, 0), (1, 1), (0, 1)):
                po = 64 * hh
                ps = psA.tile([128, 512], F32, tag="s",
                              name=f"s_{pr}_{kt_i}_{hh}_{qb}")
                nc.tensor.matmul(
                    ps,
                    kt_sb[po:po + 64, pr, kblk],
                    qt_sb[po:po + 64, pr, qb * 512:(qb + 1) * 512],
                    start=True, stop=True,
                )
                s_t[(hh, qb)] = ps
            pexp = {}
            for hh, qb in ((0, 0), (1, 0), (1, 1), (0, 1)):
                if hh == 0:   # exact exp on ScalarE -> bf16
                    pe = pexp_pool.tile([128, 512], BF16, tag="pexp",
                                        name=f"pe_{pr}_{kt_i}_{hh}_{qb}")
                    nc.scalar.activation(pe, s_t[(hh, qb)], AF.Exp,
                                         scale=float(SCALE))
                else:         # Schraudolph fast-exp on DVE -> bf16 bits
                    pe = pexp_pool.tile([128, 512], I16, tag="pexp",
                                        name=f"pe_{pr}_{kt_i}_{hh}_{qb}")
                    nc.vector.tensor_scalar(
                        pe, s_t[(hh, qb)], EXP_A, EXP_B, ALU.mult, ALU.add)
                    pe = pe.bitcast(BF16)
                pexp[(hh, qb)] = pe
            if prev is not None:
                emit_o(*prev)
            prev = (kt_i, pexp)
        emit_o(*prev)

        if pr == NP - 1:
            while pending_norm:
                emit_norm(*pending_norm.pop(0))
        for hh in range(2):
            po = 64 * hh
            for qb in range(2):
                qs = slice(qb * 512, (qb + 1) * 512)
                stg = stage_pool.tile([65, 512], F32, tag="stage",
                                      name=f"stg_{pr}_{hh}_{qb}")
                if hh == 0:
                    nc.scalar.copy(stg, o_ps[(hh, qb)])
                else:
                    nc.vector.tensor_copy(stg, o_ps[(hh, qb)])
                nc.sync.dma_start(
                    ot_sb[po:po + 64, pr, qs], stg[0:HD, :].bitcast(F32R))
                nc.sync.dma_start(
                    spair[pr][hh:hh + 1, qs], stg[HD:HD + 1, :])

        if pr < NP - 1:
            nc.vector.reciprocal_approx_fast(out=rpair[pr], in_=spair[pr])
            fl = flat_pool.tile([1, N], F32, tag="flat", name=f"fl_{pr}")
            nc.scalar.dma_start(fl, rpair[pr][1:2, :])
            pending_norm.append((pr, fl))
        else:
            # Fast tail: the proj phase waits on this pair's normalization,
            # so shorten the chain (qb-split, PE broadcast, DVE multiplies).
            fl = flat_pool.tile([1, N], F32, tag="flat", name=f"fl_{pr}")
            for qb in range(2):
                qs = slice(qb * 512, (qb + 1) * 512)
                nc.vector.reciprocal_approx_fast(
                    out=rpair[pr][:, qs], in_=spair[pr][:, qs])
                nc.sync.dma_start(fl[0:1, qs], rpair[pr][1:2, qs])
                r_ps = psA.tile([128, 512], F32, tag="s",
                                name=f"rps_{pr}_{qb}")
                nc.tensor.matmul(r_ps[0:64, :], ones[0:1, 0:HD],
                                 rpair[pr][0:1, qs], start=True, stop=True)
                nc.tensor.matmul(r_ps[64:128, :], ones[0:1, 0:HD],
                                 fl[0:1, qs], start=True, stop=True)
                dst = ot_sb[:, pr, qs]
                nc.vector.tensor_mul(out=dst, in0=dst, in1=r_ps)

    flat_pool.release()
    stage_pool.release()
    pexp_pool.release()

    # ---- phase 4: out = O @ w_proj (+ b_proj) ----
    # The kt_i = DT-1 accumulation step needs the last head pair's normalized
    # ot_sb columns, which arrive late; emit the kt_i < DT-1 partials two seq
    # tiles ahead so the in-order PE queue isn't blocked on the pr=5 epilogue.
    fout_pool = tc.alloc_tile_pool(name="fout", bufs=3)
    segs = ((0, 512), (512, 256))
    fps = {}

    def proj_partial(st):
        pool, tg = (psA, "s") if st % 4 in (0, 1) else (psB, "o")
        for sb, (noff, nw) in enumerate(segs):
            f_ps = pool.tile([128, 512], F32, tag=tg, name=f"f_{st}_{sb}")
            fps[(st, sb)] = f_ps
            for kt_i in range(DT - 1):
                nc.tensor.matmul(
                    f_ps[:, 0:nw],
                    ot_sb[:, kt_i, st * 128:(st + 1) * 128],
                    wp_sb[:, kt_i, noff:noff + nw],
                    start=(kt_i == 0), stop=False,
                    skip_group_check=True,
                )

    for st in range(4):
        proj_partial(st)
    for st in range(NT):
        fo = fout_pool.tile([128, D], F32, tag="fout", name=f"fo_{st}")
        for sb, (noff, nw) in enumerate(segs):
            seg = fps.pop((st, sb))[:, 0:nw]
            nc.tensor.matmul(
                seg,
                ot_sb[:, DT - 1, st * 128:(st + 1) * 128],
                wp_sb[:, DT - 1, noff:noff + nw],
                start=False, stop=not with_bias,
                skip_group_check=True,
            )
            if with_bias:
                nc.tensor.matmul(
                    seg,
                    ones[0:1, 0:128],
                    bp_row[0:1, noff:noff + nw],
                    start=False, stop=True,
                )
            if sb == 0:
                nc.vector.tensor_copy(fo[:, noff:noff + nw], seg)
            else:
                nc.scalar.copy(fo[:, noff:noff + nw], seg)
        if st + 4 < NT:
            proj_partial(st + 4)
        nc.sync.dma_start(out_d.ap()[st * 128:(st + 1) * 128, :], fo)

    fout_pool.release()
    late.release()
    psB.release()
    psA.release()
    top.release()


_module_cache: dict = {}


def get_module(with_bias: bool) -> bass.Bass:
    if with_bias not in _module_cache:
        _module_cache[with_bias] = build_module(with_bias)
    return _module_cache[with_bias]


def kernel(x, w_qkv, b_qkv, w_proj, b_proj):
    from concourse.bass_utils import run_bass_kernel_spmd

    x = np.ascontiguousarray(np.asarray(x, dtype=np.float32))
    w_qkv = np.ascontiguousarray(np.asarray(w_qkv, dtype=np.float32))
    b_qkv = np.ascontiguousarray(np.asarray(b_qkv, dtype=np.float32)).reshape(1, 3 * D)
    w_proj = np.ascontiguousarray(np.asarray(w_proj, dtype=np.float32))
    b_proj = np.ascontiguousarray(np.asarray(b_proj, dtype=np.float32)).reshape(1, D)

    B = x.shape[0]
    assert x.shape == (B, N, D) and B == 8, x.shape

    with_bias = bool(np.any(b_qkv) or np.any(b_proj))
    nc = get_module(with_bias)

    in_maps = [
        {
            "x": np.ascontiguousarray(x[b]),
            "w_qkv": w_qkv,
            "b_qkv": b_qkv,
            "w_proj": w_proj,
            "b_proj": b_proj,
        }
        for b in range(B)
    ]
    res = run_bass_kernel_spmd(nc, in_maps, core_ids=list(range(B)))
    kernel.last_results = res
    return np.stack([res.results[b]["out"] for b in range(B)], axis=0)

